# revision 7
# baseline (speedup 1.0000x reference)
"""Trainium2 Bass kernel for nn_ChessMoveSelector.

Full model:
    score[b,n] = board_score[b] + move_emb[b,n,:] @ wm + biases
    probs      = ragged_softmax_n(score) * mask        (mask: n < lengths[b])

The softmax over n is invariant to any per-row (per-b) additive constant.
board_score[b] (conv tower + fc + extra branch) and every bias term are
constant in n, so they cancel exactly:

    probs[b, :] = ragged_softmax_n(moves[b,n,:] @ c),   c = move_w.T @ wm

with wm = comb_w[0, BD:].  Only `moves`, `lengths`, `move_w`, `comb_w`
influence the output.  c is computed on-device (two tiny matmuls); the
masked softmax runs on the vector/scalar engines.

Sharding: pure data parallel.  B=4096 rows -> 8 cores x 512 rows.  Each
core lays its 512 rows out as [128 partitions x 4 row-groups].
"""

from contextlib import ExitStack

import numpy as np

import concourse.bass as bass
import concourse.tile as tile
from concourse import bacc, mybir
from concourse.alu_op_type import AluOpType
from concourse.bass_utils import run_bass_kernel_spmd

N_CORES = 8
B = 4096
NMAX = 64
BD, MD = 256, 128
B_LOCAL = B // N_CORES       # 512
P = 128                      # SBUF partitions
T = B_LOCAL // P             # 4 row-groups per partition

F32 = mybir.dt.float32
I32 = mybir.dt.int32

_CACHE: dict = {}


def _build_program() -> bass.Bass:
    nc = bacc.Bacc("TRN2", target_bir_lowering=False, debug=False)

    moves_d = nc.declare_dram_parameter("moves", [B_LOCAL, NMAX, 2], F32, isOutput=False)
    len_d = nc.declare_dram_parameter("lengths", [B_LOCAL], I32, isOutput=False)
    mw_d = nc.declare_dram_parameter("move_w", [MD, 2], F32, isOutput=False)
    cw_d = nc.declare_dram_parameter("comb_w", [1, BD + MD], F32, isOutput=False)
    out_d = nc.declare_dram_parameter("out", [B_LOCAL, NMAX], F32, isOutput=True)

    with tile.TileContext(nc) as tc, ExitStack() as ctx:
        pool = ctx.enter_context(tc.tile_pool(name="sbuf", bufs=1))
        psum = ctx.enter_context(tc.tile_pool(name="psum", bufs=1, space="PSUM"))

        # ---- c = move_w.T @ wm, broadcast to all 128 partitions ----
        mw = pool.tile([MD, 2], F32, tag="mw")
        nc.sync.dma_start(mw[:], mw_d.ap())
        wmt = pool.tile([MD, 1], F32, tag="wmt")
        nc.sync.dma_start(wmt[:], cw_d.ap()[:, BD:].rearrange("o m -> m o"))

        # Stage matmul operands through one engine so each matmul carries a
        # single sem wait (the ISA allows very few waits per matmul).
        stage = pool.tile([MD, 3], F32, tag="stage")
        nc.vector.tensor_copy(stage[:, 0:2], mw[:])
        nc.vector.tensor_copy(stage[:, 2:3], wmt[:])

        c_ps = psum.tile([1, 2], F32, tag="c_ps")
        nc.tensor.matmul(c_ps[:], lhsT=stage[:, 2:3], rhs=stage[:, 0:2], start=True, stop=True)

        stage2 = pool.tile([1, P + 2], F32, tag="stage2")
        nc.vector.memset(stage2[:, 0:P], 1.0)
        nc.vector.tensor_copy(stage2[:, P : P + 2], c_ps[:])
        cb_ps = psum.tile([P, 2], F32, tag="cb_ps")
        nc.tensor.matmul(
            cb_ps[:], lhsT=stage2[:, 0:P], rhs=stage2[:, P : P + 2], start=True, stop=True
        )
        cb = pool.tile([P, 2], F32, tag="cb")
        nc.scalar.copy(cb[:], cb_ps[:])

        # ---- constants: iota 0..NMAX-1 repeated over the T row-groups ----
        iota_f = pool.tile([P, T, NMAX], F32, tag="iota")
        nc.gpsimd.iota(
            iota_f[:], pattern=[[0, T], [1, NMAX]], base=0,
            channel_multiplier=0, allow_small_or_imprecise_dtypes=True,
        )

        # ---- load the shard: row b = t*P + p  ->  partition p, group t ----
        mv = pool.tile([P, T, NMAX, 2], F32, tag="mv")
        nc.sync.dma_start(mv[:], moves_d.ap().rearrange("(t p) n f -> p t n f", p=P))
        len_i = pool.tile([P, T], I32, tag="len_i")
        nc.sync.dma_start(len_i[:], len_d.ap().rearrange("(t p) -> p t", p=P))
        lenf = pool.tile([P, T], F32, tag="lenf")
        nc.vector.tensor_copy(lenf[:], len_i[:])

        # ---- mask[p,t,n] = iota < lengths ----
        mask = pool.tile([P, T, NMAX], mybir.dt.uint8, tag="mask")
        nc.vector.tensor_tensor(
            mask[:], iota_f[:], lenf[:].unsqueeze(2).broadcast_to([P, T, NMAX]),
            op=AluOpType.is_lt,
        )

        # ---- scores s = c0*moves[...,0] + c1*moves[...,1] ----
        s1 = pool.tile([P, T, NMAX], F32, tag="s1")
        nc.vector.tensor_scalar(
            s1[:], mv[:, :, :, 1], cb[:, 1:2], None, op0=AluOpType.mult
        )
        s = pool.tile([P, T, NMAX], F32, tag="s")
        nc.vector.scalar_tensor_tensor(
            s[:], in0=mv[:, :, :, 0], scalar=cb[:, 0:1], in1=s1[:],
            op0=AluOpType.mult, op1=AluOpType.add,
        )

        # ---- masked logits -> ragged softmax ----
        sm = pool.tile([P, T, NMAX], F32, tag="sm")
        nc.vector.memset(sm[:], -1e30)
        nc.vector.copy_predicated(sm[:], mask[:], s[:])

        rmax = pool.tile([P, T], F32, tag="rmax")
        nc.vector.tensor_reduce(rmax[:], sm[:], axis=mybir.AxisListType.X, op=AluOpType.max)

        smx = pool.tile([P, T, NMAX], F32, tag="smx")
        nc.vector.tensor_tensor(
            smx[:], sm[:], rmax[:].unsqueeze(2).broadcast_to([P, T, NMAX]),
            op=AluOpType.subtract,
        )
        e = pool.tile([P, T, NMAX], F32, tag="e")
        nc.scalar.activation(e[:], smx[:], mybir.ActivationFunctionType.Exp)

        ssum = pool.tile([P, T], F32, tag="ssum")
        nc.vector.tensor_reduce(ssum[:], e[:], axis=mybir.AxisListType.X, op=AluOpType.add)
        rec = pool.tile([P, T], F32, tag="rec")
        nc.vector.reciprocal(rec[:], ssum[:])

        outp = pool.tile([P, T, NMAX], F32, tag="outp")
        nc.vector.tensor_tensor(
            outp[:], e[:], rec[:].unsqueeze(2).broadcast_to([P, T, NMAX]),
            op=AluOpType.mult,
        )
        nc.sync.dma_start(out_d.ap().rearrange("(t p) n -> p t n", p=P), outp[:])

    nc.compile()
    return nc


def _get_program() -> bass.Bass:
    if "nc" not in _CACHE:
        _CACHE["nc"] = _build_program()
    return _CACHE["nc"]


def kernel(**inputs: np.ndarray) -> np.ndarray:
    moves = np.ascontiguousarray(np.asarray(inputs["moves"], dtype=np.float32))
    lengths = np.ascontiguousarray(np.asarray(inputs["lengths"], dtype=np.int32))
    move_w = np.ascontiguousarray(np.asarray(inputs["move_w"], dtype=np.float32))
    comb_w = np.ascontiguousarray(np.asarray(inputs["comb_w"], dtype=np.float32))

    nc = _get_program()
    in_maps = [
        {
            "moves": moves[i * B_LOCAL : (i + 1) * B_LOCAL],
            "lengths": lengths[i * B_LOCAL : (i + 1) * B_LOCAL],
            "move_w": move_w,
            "comb_w": comb_w,
        }
        for i in range(N_CORES)
    ]
    res = run_bass_kernel_spmd(nc, in_maps, core_ids=list(range(N_CORES)))
    return np.concatenate([res.results[i]["out"] for i in range(N_CORES)], axis=0)


# revision 10
# speedup vs baseline: 1.4255x; 1.4255x over previous
"""Trainium2 Bass kernel for nn_ChessMoveSelector (B=4096, NMAX=64).

Reference model:
    board_emb = relu(conv2(relu(conv1(board))).flat @ fc_w.T + fc_b)
                + extra @ extra_w.T + extra_b                      # [B, 256]
    move_emb  = moves @ move_w.T + move_b                          # [B, 64, 128]
    score     = board_emb @ wb.T + move_emb @ wm.T + comb_b        # [B, 64]
    probs     = ragged_softmax_n(score) * (n < lengths)

Key algebraic identity: the softmax runs over n (the move axis), and
board_emb/extra/all biases contribute a per-row constant that cancels
exactly in the softmax.  The output therefore reduces to

    probs[b, :] = ragged_softmax_n(moves[b, n, :] @ c),  c = move_w.T @ wm

with wm = comb_w[0, 256:].  Only moves, lengths, move_w and comb_w can
affect the output; the conv tower is dead code.  (Verified: max
elementwise relative error vs the full reference is ~2e-5, dominated by
fp32 rounding, not the rewrite.)

Device structure (raw Bacc, manual semaphores, no TileContext):
  * Pure data parallel: B=4096 rows -> 8 cores x 512 rows; each core
    lays rows out as [128 partitions x 4 row-groups], b_local = 4p + t,
    so every partition reads one contiguous 2KB chunk of moves.
  * move_w/wm are replicated across partitions on the host (layout
    only — the sharding hint's "replicate the tiny parameter set") and
    c is computed per-partition on the vector engine.
  * The exp uses a per-partition bias -63*(|c0|+|c1|) <= -max_n score
    (moves are bounded in [0, 63]) instead of a row max — saves two
    serial vector ops.
  * The ragged mask is built from an iota constant baked into the NEFF
    and applied multiplicatively after the exp, computed on the vector
    engine while the scalar engine runs the exp.
  * Large DMAs are split across the two HWDGE rings (sync + scalar
    engines) to double per-stream DMA throughput.
"""

from contextlib import ExitStack

import numpy as np

import concourse.bass as bass
from concourse import bacc, mybir
from concourse.alu_op_type import AluOpType
from concourse.bass_utils import run_bass_kernel_spmd

N_CORES = 8
B = 4096
NMAX = 64
BD, MD = 256, 128
B_LOCAL = B // N_CORES       # 512
P = 128                      # SBUF partitions
T = B_LOCAL // P             # 4 row-groups per partition

F32 = mybir.dt.float32
I32 = mybir.dt.int32

_CACHE: dict = {}


def _build_program() -> bass.Bass:
    nc = bacc.Bacc("TRN2", target_bir_lowering=False, debug=False)

    moves_d = nc.declare_dram_parameter("moves", [B_LOCAL, NMAX, 2], F32, isOutput=False)
    len_d = nc.declare_dram_parameter("lengths", [B_LOCAL], I32, isOutput=False)
    wrep_d = nc.declare_dram_parameter("wrep", [P, 3, MD], F32, isOutput=False)
    out_d = nc.declare_dram_parameter("out", [B_LOCAL, NMAX], F32, isOutput=True)

    iota_np = np.broadcast_to(np.arange(NMAX, dtype=np.int32), (P, T, NMAX))
    iota_c = nc.inline_tensor(np.ascontiguousarray(iota_np), name="iota_c")

    with ExitStack() as ctx:
        en = ctx.enter_context

        mv = en(nc.sbuf_tensor("mv", [P, T, NMAX, 2], F32)).ap()
        len_i = en(nc.sbuf_tensor("len_i", [P, T], I32)).ap()
        iota_i = en(nc.sbuf_tensor("iota_i", [P, T, NMAX], I32)).ap()
        wrep = en(nc.sbuf_tensor("wrep_sb", [P, 3, MD], F32)).ap()
        prod = en(nc.sbuf_tensor("prod", [P, 2, MD], F32)).ap()
        cb = en(nc.sbuf_tensor("cb", [P, 2], F32)).ap()
        csum = en(nc.sbuf_tensor("csum", [P, 1], F32)).ap()
        m63 = en(nc.sbuf_tensor("m63", [P, 1], F32)).ap()
        minv = en(nc.sbuf_tensor("minv", [P, T, NMAX], F32)).ap()
        s1 = en(nc.sbuf_tensor("s1", [P, T, NMAX], F32)).ap()
        sm = en(nc.sbuf_tensor("sm", [P, T, NMAX], F32)).ap()
        e = en(nc.sbuf_tensor("e", [P, T, NMAX], F32)).ap()
        em = en(nc.sbuf_tensor("em", [P, T, NMAX], F32)).ap()
        ssum = en(nc.sbuf_tensor("ssum", [P, T], F32)).ap()
        rec = en(nc.sbuf_tensor("rec", [P, T], F32)).ap()
        outp = en(nc.sbuf_tensor("outp", [P, T, NMAX], F32)).ap()

        d_w = en(nc.semaphore("d_w"))
        d_iota = en(nc.semaphore("d_iota"))
        d_len = en(nc.semaphore("d_len"))
        d_mv = en(nc.semaphore("d_mv"))
        d_out = en(nc.semaphore("d_out"))
        s_dve = en(nc.semaphore("s_dve"))
        s_act = en(nc.semaphore("s_act"))

        with nc.Block() as block:
            HP = P // 2  # split big transfers across the two HWDGE rings
            mv_r = moves_d.ap().rearrange("(p t) n f -> p t n f", p=P)
            out_r = out_d.ap().rearrange("(p t) n -> p t n", p=P)

            @block.scalar
            def _(act: bass.BassEngine):
                act.dma_start(wrep[HP:, :, :], wrep_d.ap()[HP:, :, :]).then_inc(d_w, 16)
                act.dma_start(mv[HP:], mv_r[HP:]).then_inc(d_mv, 16)
                act.dma_start(iota_i, iota_c.ap()).then_inc(d_iota, 16)
                act.dma_start(len_i, len_d.ap().rearrange("(p t) -> p t", p=P)).then_inc(
                    d_len, 16
                )
                # m63 = -63 * csum, computed on ACT (it consumes it as bias)
                act.mul(m63, csum, -63.0)._wait_ge(s_dve, 3).then_inc(s_act, 1)
                act.wait_ge(s_act, 1)
                act.activation(
                    e, sm, mybir.ActivationFunctionType.Exp, bias=m63
                )._wait_ge(s_dve, 5).then_inc(s_act, 1)
                act.dma_start(out_r[HP:], outp[HP:])._wait_ge(s_dve, 10).then_inc(
                    d_out, 16
                )

            @block.sync
            def _(sp: bass.BassEngine):
                sp.dma_start(wrep[:HP, :, :], wrep_d.ap()[:HP, :, :]).then_inc(d_w, 16)
                sp.dma_start(mv[:HP], mv_r[:HP]).then_inc(d_mv, 16)
                sp.dma_start(out_r[:HP], outp[:HP])._wait_ge(s_dve, 10).then_inc(
                    d_out, 16
                )
                # final gate: both output halves landed before the NEFF ends
                sp.wait_ge(d_out, 32)

            @block.vector
            def _(dve: bass.BassEngine):
                # c[f] = sum_m move_w[m, f] * wm[m], computed on every partition
                dve.tensor_tensor(
                    prod, wrep[:, 0:2, :],
                    wrep[:, 2, :].unsqueeze(1).broadcast_to([P, 2, MD]),
                    op=AluOpType.mult,
                )._wait_ge(d_w, 32).then_inc(s_dve, 1)
                dve.tensor_reduce(
                    cb, prod, axis=mybir.AxisListType.X, op=AluOpType.add
                )._wait_ge(s_dve, 1).then_inc(s_dve, 1)
                # exp bias input: csum = |c0|+|c1|  (ACT scales by -63)
                dve.tensor_reduce(
                    csum, cb, axis=mybir.AxisListType.X, op=AluOpType.add,
                    apply_absolute_value=True,
                )._wait_ge(s_dve, 2).then_inc(s_dve, 1)
                # scores (no masking needed pre-exp: bias keeps exp <= 1)
                dve.wait_ge(d_mv, 32)
                dve.tensor_scalar(
                    s1, mv[:, :, :, 1], cb[:, 1:2], None, op0=AluOpType.mult
                )._wait_ge(s_dve, 3).then_inc(s_dve, 1)
                dve.scalar_tensor_tensor(
                    sm, in0=mv[:, :, :, 0], scalar=cb[:, 0:1], in1=s1,
                    op0=AluOpType.mult, op1=AluOpType.add,
                )._wait_ge(s_dve, 4).then_inc(s_dve, 1)
                # ragged mask, computed while ACT runs the exp
                dve.wait_ge(d_iota, 16)
                dve.wait_ge(d_len, 16)
                dve.tensor_tensor(
                    minv, iota_i, len_i.unsqueeze(2).broadcast_to([P, T, NMAX]),
                    op=AluOpType.is_lt,
                )._wait_ge(s_dve, 5).then_inc(s_dve, 1)
                dve.wait_ge(s_act, 2)
                dve.tensor_tensor(em, e, minv, op=AluOpType.mult)._wait_ge(
                    s_dve, 6
                ).then_inc(s_dve, 1)
                dve.tensor_reduce(
                    ssum, em, axis=mybir.AxisListType.X, op=AluOpType.add
                )._wait_ge(s_dve, 7).then_inc(s_dve, 1)
                dve.reciprocal(rec, ssum)._wait_ge(s_dve, 8).then_inc(s_dve, 1)
                dve.tensor_tensor(
                    outp, em, rec.unsqueeze(2).broadcast_to([P, T, NMAX]),
                    op=AluOpType.mult,
                )._wait_ge(s_dve, 9).then_inc(s_dve, 1)

    nc.compile()
    return nc


def _get_program() -> bass.Bass:
    if "nc" not in _CACHE:
        _CACHE["nc"] = _build_program()
    return _CACHE["nc"]


def kernel(**inputs: np.ndarray) -> np.ndarray:
    moves = np.ascontiguousarray(np.asarray(inputs["moves"], dtype=np.float32))
    lengths = np.ascontiguousarray(np.asarray(inputs["lengths"], dtype=np.int32))
    move_w = np.asarray(inputs["move_w"], dtype=np.float32)
    comb_w = np.asarray(inputs["comb_w"], dtype=np.float32)

    # replicate the tiny parameter set across partitions (layout only)
    wrep = np.empty((P, 3, MD), dtype=np.float32)
    wrep[:, 0, :] = move_w[:, 0][None, :]
    wrep[:, 1, :] = move_w[:, 1][None, :]
    wrep[:, 2, :] = comb_w[0, BD:][None, :]

    nc = _get_program()
    in_maps = [
        {
            "moves": moves[i * B_LOCAL : (i + 1) * B_LOCAL],
            "lengths": lengths[i * B_LOCAL : (i + 1) * B_LOCAL],
            "wrep": wrep,
        }
        for i in range(N_CORES)
    ]
    res = run_bass_kernel_spmd(nc, in_maps, core_ids=list(range(N_CORES)))
    return np.concatenate([res.results[i]["out"] for i in range(N_CORES)], axis=0)
